# revision 8
# baseline (speedup 1.0000x reference)
"""Bahdanau attention kernel for Trainium2 (Bass/Tile), SPMD over 8 NeuronCores.

Reference computation (per example b):
    q_proj = query @ W1_k + W1_b                     # [U]
    v_proj = values @ W2_k + W2_b                    # [T, U]
    score  = tanh(q_proj + v_proj) @ V_k + V_b       # [T]
    attn   = softmax(score)                          # over T
    out    = sum_t attn[t] * values[t]               # [D]

Sharding: data-parallel over batch B=32 -> 4 examples per core; params
replicated. No collectives.

Per-core dataflow (all heavy matmuls in bf16, fp32 accumulation):
  - values are loaded naturally ([t, d] tiles, cast fp32->bf16 during DMA),
    transposed on-chip via PE transpose to valuesT [d, t] tiles, which feed
    the W2 matmul as the moving operand.
  - tanh(q_proj + W1_b + W2_b + v_proj) is fused into one ScalarE activation
    with a per-partition bias (v_proj computed transposed: [u, t]).
  - score = V^T tanh(...) via M=1 matmuls accumulated over u-tiles.
  - V_b is dropped: softmax is invariant to constant shifts.
  - softmax without max subtraction (|score| <= sum|V_k| ~ 16 << fp32 range).
  - score is broadcast to 128 partitions via a K=1 ones-matmul, exp'd on
    ScalarE with a fused accumulated sum, and the context reduction
    context[d] = sum_t attn[t]*valuesT[d,t] runs on VectorE via
    tensor_tensor_reduce over the transposed values already in SBUF.
"""

import sys

_REPO = "/opt/trn_rl_repo"
if _REPO not in sys.path:
    sys.path.insert(0, _REPO)

import numpy as np
from contextlib import ExitStack

import concourse.bass as bass
import concourse.tile as tile
from concourse import mybir
from concourse.bass_utils import run_bass_kernel_spmd
from concourse.masks import make_identity

B, T, D, U = 32, 2048, 1024, 1024
NCORES = 8
BL = B // NCORES  # 4 examples per core

P = 128
KT = D // P       # 8 contraction tiles over d
UT = U // P       # 8 tiles over u
NQ = 4            # T quarters
QT = T // NQ      # 512
TCH = QT // P     # 4 [128-row] t-chunks per quarter

F32 = mybir.dt.float32
BF16 = mybir.dt.bfloat16
ADD = mybir.AluOpType.add
MULT = mybir.AluOpType.mult
AF = mybir.ActivationFunctionType
AX = mybir.AxisListType


def _emit(ctx: ExitStack, tc: tile.TileContext, q, v, w1k, w1b, w2k, w2b, vk, out):
    nc = tc.nc

    singles = ctx.enter_context(tc.tile_pool(name="singles", bufs=1))
    natp = ctx.enter_context(tc.tile_pool(name="natp", bufs=2))
    vtp = ctx.enter_context(tc.tile_pool(name="vtp", bufs=9))
    tanhp = ctx.enter_context(tc.tile_pool(name="tanhp", bufs=3))
    attnp = ctx.enter_context(tc.tile_pool(name="attnp", bufs=2))
    scrp = ctx.enter_context(tc.tile_pool(name="scrp", bufs=2))
    smallp = ctx.enter_context(tc.tile_pool(name="smallp", bufs=4))
    scorep = ctx.enter_context(tc.tile_pool(name="scorep", bufs=2))
    tpsum = ctx.enter_context(tc.tile_pool(name="tpsum", bufs=2, space="PSUM"))
    vpsum = ctx.enter_context(tc.tile_pool(name="vpsum", bufs=2, space="PSUM"))
    spsum = ctx.enter_context(tc.tile_pool(name="spsum", bufs=2, space="PSUM"))

    # ---- replicated parameters -------------------------------------------
    w2_sb = singles.tile([P, KT, U], BF16, tag="w2", name="w2_sb")
    nc.gpsimd.dma_start(out=w2_sb[:], in_=w2k.rearrange("(kt p) u -> p kt u", p=P))

    w1_sb = singles.tile([P, KT, U], F32, tag="w1", name="w1_sb")
    nc.sync.dma_start(out=w1_sb[:], in_=w1k.rearrange("(kt p) u -> p kt u", p=P))

    v_sb = singles.tile([P, UT], BF16, tag="vk", name="v_sb")
    nc.gpsimd.dma_start(out=v_sb[:], in_=vk.rearrange("(ut p) one -> p (ut one)", p=P))

    qT_sb = singles.tile([P, KT, BL], F32, tag="qT", name="qT_sb")
    for kt in range(KT):
        nc.sync.dma_start(
            out=qT_sb[:, kt, :],
            in_=q[:, kt * P : (kt + 1) * P].rearrange("b p -> p b"),
        )

    w1b_sb = singles.tile([P, UT], F32, tag="w1b", name="w1b_sb")
    nc.sync.dma_start(out=w1b_sb[:], in_=w1b.rearrange("(ut p) -> p ut", p=P))

    w2b_sb = singles.tile([P, UT], F32, tag="w2b", name="w2b_sb")
    nc.sync.dma_start(out=w2b_sb[:], in_=w2b.rearrange("(ut p) -> p ut", p=P))

    ident = singles.tile([P, P], BF16, tag="ident", name="ident")
    make_identity(nc, ident[:])
    ident32 = singles.tile([P, P], F32, tag="ident32", name="ident32")
    make_identity(nc, ident32[:])

    ones_sb = singles.tile([1, P], BF16, tag="ones", name="ones_sb")
    nc.vector.memset(ones_sb[:], 1.0)

    # context accumulator for all local examples, [d_inner, b, d_tile]
    ctx_all = singles.tile([P, BL, KT], F32, tag="ctxall", name="ctx_all")

    # ---- q_proj + biases: qb[u, b] = query@W1 + W1_b + W2_b --------------
    qb_sb = singles.tile([P, UT, BL], F32, tag="qb", name="qb_sb")
    for ut in range(UT):
        psq = vpsum.tile([P, QT], F32, tag="vp", name="psq")
        for kt in range(KT):
            nc.tensor.matmul(
                psq[:, :BL],
                lhsT=w1_sb[:, kt, ut * P : (ut + 1) * P],
                rhs=qT_sb[:, kt, :],
                start=(kt == 0),
                stop=(kt == KT - 1),
            )
        nc.vector.tensor_scalar(
            out=qb_sb[:, ut, :],
            in0=psq[:, :BL],
            scalar1=w1b_sb[:, ut : ut + 1],
            scalar2=w2b_sb[:, ut : ut + 1],
            op0=ADD,
            op1=ADD,
        )

    # ---- values load + on-chip transpose ---------------------------------
    vt_tiles = {}

    def load_transpose_quarter(b, qi):
        natt = natp.tile([P, TCH, D], BF16, tag="nat", name="natt")
        nc.gpsimd.dma_start(
            out=natt[:],
            in_=v[b, qi * QT : (qi + 1) * QT, :].rearrange("(c p) d -> p c d", p=P),
        )
        vt = vtp.tile([P, KT, QT], BF16, tag="vt", name="vt")
        for c in range(TCH):
            for kt in range(KT):
                pst = tpsum.tile([P, P], BF16, tag="tp", name="pst")
                nc.tensor.transpose(pst[:], natt[:, c, kt * P : (kt + 1) * P], ident[:])
                nc.vector.tensor_copy(out=vt[:, kt, c * P : (c + 1) * P], in_=pst[:])
        vt_tiles[(b, qi)] = vt

    for qi in range(NQ):
        load_transpose_quarter(0, qi)

    # ---- main per-example pipeline ---------------------------------------
    for b in range(BL):
        score_sb = scorep.tile([1, T], BF16, tag="score", name="score_sb")
        for c in range(NQ):
            if b + 1 < BL:
                load_transpose_quarter(b + 1, c)
            ps_sc = spsum.tile([P, QT], F32, tag="sm", name="ps_sc")
            vt = vt_tiles[(b, c)]
            prev = None
            for ut in range(UT):
                psv = vpsum.tile([P, QT], F32, tag="vp", name="psv")
                for kt in range(KT):
                    nc.tensor.matmul(
                        psv[:],
                        lhsT=w2_sb[:, kt, ut * P : (ut + 1) * P],
                        rhs=vt[:, kt, :],
                        start=(kt == 0),
                        stop=(kt == KT - 1),
                    )
                th = tanhp.tile([P, QT], BF16, tag="th", name="th")
                nc.scalar.activation(
                    out=th[:], in_=psv[:], func=AF.Tanh, bias=qb_sb[:, ut, b : b + 1]
                )
                # defer the score matmul by one u-tile so tanh can complete
                # while the next u-tile's matmuls stream
                if prev is not None:
                    put, pth = prev
                    nc.tensor.matmul(
                        ps_sc[:1, :],
                        lhsT=v_sb[:, put : put + 1],
                        rhs=pth[:],
                        start=(put == 0),
                        stop=False,
                    )
                prev = (ut, th)
            put, pth = prev
            nc.tensor.matmul(
                ps_sc[:1, :],
                lhsT=v_sb[:, put : put + 1],
                rhs=pth[:],
                start=False,
                stop=True,
            )
            nc.scalar.copy(out=score_sb[:, c * QT : (c + 1) * QT], in_=ps_sc[:1, :])

        # softmax over T (no max subtraction; |score| <= sum|V_k| ~ 16)
        attn = attnp.tile([P, T], F32, tag="attn", name="attn")
        sep = smallp.tile([P, NQ], F32, tag="sep", name="sep")
        for c in range(NQ):
            psb = spsum.tile([P, QT], F32, tag="sm", name="psb")
            nc.tensor.matmul(
                psb[:], lhsT=ones_sb[:], rhs=score_sb[:, c * QT : (c + 1) * QT],
                start=True, stop=True,
            )
            nc.scalar.activation(
                out=attn[:, c * QT : (c + 1) * QT],
                in_=psb[:],
                func=AF.Exp,
                accum_out=sep[:, c : c + 1],
            )
        sumexp = smallp.tile([P, 1], F32, tag="sumexp", name="sumexp")
        nc.vector.reduce_sum(out=sumexp[:], in_=sep[:], axis=AX.X)
        rcp = smallp.tile([P, 1], F32, tag="rcp", name="rcp")
        nc.vector.reciprocal(out=rcp[:], in_=sumexp[:])

        # context[d] = (1/sumexp) * sum_t exp(score_t) * valuesT[d, t]
        ctxp_t = smallp.tile([P, KT, NQ], F32, tag="ctxp", name="ctxp_t")
        for qi in range(NQ):
            vt = vt_tiles.pop((b, qi))
            for dt in range(KT):
                scr = scrp.tile([P, QT], F32, tag="scr", name="scr")
                nc.vector.tensor_mul(
                    out=scr[:],
                    in0=vt[:, dt, :],
                    in1=attn[:, qi * QT : (qi + 1) * QT],
                )
                nc.vector.reduce_sum(
                    out=ctxp_t[:, dt, qi : qi + 1], in_=scr[:], axis=AX.X
                )
        ctxs = smallp.tile([P, KT], F32, tag="ctxs", name="ctxs")
        nc.vector.reduce_sum(out=ctxs[:], in_=ctxp_t[:], axis=AX.X)
        nc.vector.tensor_scalar_mul(
            out=ctx_all[:, b, :], in0=ctxs[:], scalar1=rcp[:]
        )

    # ---- write out: transpose [d_inner, (b, d_tile)] -> contiguous rows --
    pso = tpsum.tile([BL * KT, P], F32, tag="tpo", name="pso")
    nc.tensor.transpose(pso[:], ctx_all.rearrange("p b k -> p (b k)"), ident32[:])
    ctxT = smallp.tile([BL * KT, P], F32, tag="ctxT", name="ctxT")
    nc.vector.tensor_copy(out=ctxT[:], in_=pso[:])
    nc.sync.dma_start(out=out.rearrange("b (dt p) -> (b dt) p", p=P), in_=ctxT[:])


def _split_multi_waits(nc: bass.Bass) -> int:
    """The walrus build here accepts only ONE semaphore wait per instruction;
    hoist extra waits onto single-wait NoOps preceding the instruction (same
    engine, in-order, so semantics are preserved)."""
    n_split = 0
    for f in nc.m.functions:
        for b in f.blocks:
            il = b.instructions
            out, changed = [], False
            for inst in il:
                si = inst.sync_info
                waits = list(si.on_wait) if (si and si.on_wait) else []
                if len(waits) > 1:
                    changed = True
                    n_split += 1
                    for j, w in enumerate(waits[:-1]):
                        out.append(
                            mybir.InstNoOp(
                                name=f"{inst.name}.sw{j}",
                                engine=inst.engine,
                                ins=[],
                                outs=[],
                                sync_info=mybir.SyncInfo(on_wait=[w], on_update=[]),
                            )
                        )
                    inst.sync_info = mybir.SyncInfo(
                        on_wait=[waits[-1]], on_update=list(si.on_update or [])
                    )
                out.append(inst)
            if changed:
                il[:] = out
    return n_split


def build_program(split_waits: bool = True) -> bass.Bass:
    nc = bass.Bass("TRN2", target_bir_lowering=False, debug=False, num_devices=NCORES)
    q_h = nc.dram_tensor("query", [BL, D], F32, kind="ExternalInput")
    v_h = nc.dram_tensor("values", [BL, T, D], F32, kind="ExternalInput")
    w1k_h = nc.dram_tensor("W1_k", [D, U], F32, kind="ExternalInput")
    w1b_h = nc.dram_tensor("W1_b", [U], F32, kind="ExternalInput")
    w2k_h = nc.dram_tensor("W2_k", [D, U], F32, kind="ExternalInput")
    w2b_h = nc.dram_tensor("W2_b", [U], F32, kind="ExternalInput")
    vk_h = nc.dram_tensor("V_k", [U, 1], F32, kind="ExternalInput")
    out_h = nc.dram_tensor("context", [BL, D], F32, kind="ExternalOutput")
    with tile.TileContext(nc) as tc:
        with ExitStack() as ctx:
            _emit(
                ctx, tc,
                q_h.ap(), v_h.ap(),
                w1k_h.ap(), w1b_h.ap(), w2k_h.ap(), w2b_h.ap(), vk_h.ap(),
                out_h.ap(),
            )
    if split_waits:
        _split_multi_waits(nc)
    return nc


_PROGRAM = None


def _get_program() -> bass.Bass:
    global _PROGRAM
    if _PROGRAM is None:
        _PROGRAM = build_program()
    return _PROGRAM


def make_in_maps(inputs: dict) -> list[dict]:
    f32 = lambda a: np.ascontiguousarray(np.asarray(a), dtype=np.float32)
    query = f32(inputs["query"])
    values = f32(inputs["values"])
    shared = {
        "W1_k": f32(inputs["W1_k"]),
        "W1_b": f32(inputs["W1_b"]),
        "W2_k": f32(inputs["W2_k"]),
        "W2_b": f32(inputs["W2_b"]),
        "V_k": f32(inputs["V_k"]),
    }
    in_maps = []
    for c in range(NCORES):
        sl = slice(c * BL, (c + 1) * BL)
        in_maps.append({"query": query[sl], "values": values[sl], **shared})
    return in_maps


def kernel(**inputs) -> np.ndarray:
    nc = _get_program()
    res = run_bass_kernel_spmd(nc, make_in_maps(inputs), list(range(NCORES))).results
    return np.concatenate([res[c]["context"] for c in range(NCORES)], axis=0)


if __name__ == "__main__":
    # smoke test with random data against a numpy reference
    rng = np.random.default_rng(0)
    inputs = {
        "query": rng.standard_normal((B, D), dtype=np.float32),
        "values": rng.standard_normal((B, T, D), dtype=np.float32),
        "W1_k": (rng.standard_normal((D, U)) * 0.02).astype(np.float32),
        "W1_b": np.zeros(U, np.float32),
        "W2_k": (rng.standard_normal((D, U)) * 0.02).astype(np.float32),
        "W2_b": np.zeros(U, np.float32),
        "V_k": (rng.standard_normal((U, 1)) * 0.02).astype(np.float32),
        "V_b": np.zeros(1, np.float32),
    }
    out = kernel(**inputs)
    print(out.shape, out.dtype)
